# revision 7
# baseline (speedup 1.0000x reference)
"""MelSpectrogramNet on 8 TRN2 NeuronCores (Bass/Tile), data-parallel over batch.

Math (per batch item):
  stft[f,t]  = (sum_k x[256t+k]*wc[f,k])^2 + (sum_k x[256t+k]*ws[f,k])^2
  mel        = mel_w @ stft
  x_db       = 10*log10(max(mel, 1e-10));  x_db = max(x_db, max_all(x_db)-80)
  out        = (x_db + 25) / 80

Folded DFT (the key trick): the hann window is exactly symmetric
(w[k] = w[2047-k]), so with j = k - 1023.5 the windowed DFT row is
w*cos(theta_f*j + phi_f). Folding x about the window center into
  e_m(t) = x[256t+1024+m] + x[256t+1023-m]
  o_m(t) = x[256t+1024+m] - x[256t+1023-m]        (m in [0,1024))
gives  cosDFT = cos(phi)C - sin(phi)S,  sinDFT = sin(phi)C + cos(phi)S with
  C_f = sum_m W~c[m,f] e_m,   S_f = sum_m W~s[m,f] o_m
and the power is phi-free:  power_f = C_f^2 + S_f^2.
=> the tensor-engine contraction halves (K=1024 per transform instead of
2x K=2048), which matters because the PE is the bottleneck (GPIO power
throttle caps it at 13/16 duty; the f32r version already ran at ~96% of
that throttled roofline).

Device mapping:
  - x is de-interleaved by 128-column parity into C2[r, par, u] =
    x[256u+128par+r] plus a partition-reversed copy C2R[r,...] =
    C2[127-r,...]; the DVE then computes each 128-row m-chunk of e/o as a
    single tensor_tensor add/sub of two contiguous slices (hidden under
    the matmuls of the previous tile).
  - all matmul operands are bf16 (measured end-to-end rel err ~5e-3 vs
    the 2e-2 gate); PSUM accumulation is fp32.
  - Nyquist: C_1024 = 0 exactly and the S-weight column for f=0 is exactly
    zero, so the S weights carry w~*(-1)^m (the Nyquist sine row) in the
    f=0 slot. Then stft[0] = C_0^2 + S_nyq^2; the mel weight column for
    f=0 is swapped to mel_w[:,1024] and a K=1 rank-1 matmul with
    (mel_w[:,0]-mel_w[:,1024]) x C_0^2 repairs the difference.
  - top_db clamp in linear space: pass 1 keeps out_pre in SBUF and the
    per-core max of mel; after gpsimd partition_all_reduce +
    AllReduce(max), the fixup applies out = max(out_pre, o_thr) in-place
    and DMAs straight out — no DRAM round-trip in the tail.
"""
import sys

sys.path.insert(0, "/opt/trn_rl_repo")

import ml_dtypes
import numpy as np

from concourse import bacc, bass_isa, mybir, tile
from concourse.bass_utils import run_bass_kernel_spmd

dt = mybir.dt
AF = mybir.ActivationFunctionType
ALU = mybir.AluOpType

NCORES = 8
B, T = 32, 221184
WIN, HOP = 2048, 256
FRAMES = (T - WIN) // HOP + 1  # 857
NMEL = 128
BPC = B // NCORES  # 4
UCOLS = T // 256  # 864 columns of 128 per parity
NFC = 8  # f-chunks of 128 (f = 0..1023); f=1024 (Nyquist) folded into S f=0
NMC = 8  # m-chunks of 128 (folded window half, m = 0..1023)
# Second tile overlaps the first by 3 frames so its width is a multiple of 4;
# overlapped frames are recomputed with identical values.
T_TILES = [(0, 512), (FRAMES - 348, 348)]
FIX_TILES = [(0, 512), (512, FRAMES - 512)]  # non-overlapping, for the fixup
C_LOG = 10.0 / float(np.log(10.0))  # 10*log10(x) = C_LOG * ln(x)
AMIN = 1e-10
TOPDB_LIN = 1e-8  # 10**(-80/10)

_compiled = {}


def _build_nc():
    nc = bacc.Bacc(
        "TRN2", target_bir_lowering=False, debug=False, num_devices=NCORES
    )

    c2_d = nc.dram_tensor("c2", [BPC, 128, 2, UCOLS], dt.bfloat16, kind="ExternalInput")
    c2r_d = nc.dram_tensor(
        "c2r", [BPC, 128, 2, UCOLS], dt.bfloat16, kind="ExternalInput"
    )
    wc_d = nc.dram_tensor("wc", [128, NFC, NMC, 128], dt.bfloat16, kind="ExternalInput")
    ws_d = nc.dram_tensor("ws", [128, NFC, NMC, 128], dt.bfloat16, kind="ExternalInput")
    melT_d = nc.dram_tensor("melT", [128, NFC, NMEL], dt.bfloat16, kind="ExternalInput")
    melnyq_d = nc.dram_tensor("melnyq", [1, NMEL], dt.bfloat16, kind="ExternalInput")
    out_d = nc.dram_tensor("out", [BPC, NMEL, FRAMES], dt.float32, kind="ExternalOutput")

    with tile.TileContext(nc) as tc:
        with (
            tc.tile_pool(name="sbw", bufs=1) as sbw,
            tc.tile_pool(name="sbeo", bufs=2) as sbeo,
            tc.tile_pool(name="sbe", bufs=2) as sbe,
            tc.tile_pool(name="psCS", bufs=3, space="PSUM") as psCS,
            tc.tile_pool(name="psM", bufs=2, space="PSUM") as psM,
            tc.tile_pool(name="dram", bufs=1, space="DRAM") as dram,
        ):
            # persistent SBUF tensors
            c2s, c2rs, outp = [], [], []
            for b in range(BPC):
                c2s.append(sbw.tile([128, 2, UCOLS], dt.bfloat16, name=f"c2_{b}"))
                c2rs.append(sbw.tile([128, 2, UCOLS], dt.bfloat16, name=f"c2r_{b}"))
                outp.append(sbw.tile([128, FRAMES], dt.float32, name=f"outp_{b}"))
            wc_t = [sbw.tile([128, NMC, 128], dt.bfloat16, name=f"wc{fc}") for fc in range(NFC)]
            ws_t = [sbw.tile([128, NMC, 128], dt.bfloat16, name=f"ws{fc}") for fc in range(NFC)]
            melT_t = sbw.tile([128, NFC, NMEL], dt.bfloat16, name="melT_t")
            melnyq_t = sbw.tile([1, NMEL], dt.bfloat16, name="melnyq_t")
            nslots = BPC * len(T_TILES)
            maxslots = sbw.tile([128, nslots], dt.float32, name="maxslots")

            # ---- input DMAs: b=0 slices needed by the first tile go first.
            # fold mc=0 needs c2 parity 0 + c2r parity 1, so those two land
            # first on separate queues.
            nc.gpsimd.dma_start(c2s[0][:, 0, 0:520], c2_d.ap()[0][:, 0, 0:520])
            nc.gpsimd.dma_start(c2rs[0][:, 1, 0:520], c2r_d.ap()[0][:, 1, 0:520])
            nc.gpsimd.dma_start(c2s[0][:, 1, 0:520], c2_d.ap()[0][:, 1, 0:520])
            nc.gpsimd.dma_start(c2rs[0][:, 0, 0:520], c2r_d.ap()[0][:, 0, 0:520])
            # fc=0/1 weights split across sync/scalar queues so the first
            # matmuls are never DMA-starved; melT comes later (first needed
            # ~8us in), then the remaining f-chunks alternate queues.
            nc.sync.dma_start(wc_t[0][:, 0:4], wc_d.ap()[:, 0, 0:4])
            nc.scalar.dma_start(wc_t[0][:, 4:], wc_d.ap()[:, 0, 4:])
            nc.sync.dma_start(ws_t[0][:, 0:4], ws_d.ap()[:, 0, 0:4])
            nc.scalar.dma_start(ws_t[0][:, 4:], ws_d.ap()[:, 0, 4:])
            nc.sync.dma_start(wc_t[1][:], wc_d.ap()[:, 1])
            nc.scalar.dma_start(ws_t[1][:], ws_d.ap()[:, 1])
            nc.sync.dma_start(melnyq_t[:], melnyq_d.ap())

            # Warm up the collective engine while the DFT runs so the real
            # AllReduce at the end starts with rings already configured.
            ccw_in = dram.tile([1, 128], dt.float32, name="ccw_in")
            ccw_out = dram.tile([1, 128], dt.float32, name="ccw_out")
            nc.gpsimd.collective_compute(
                "AllReduce",
                ALU.max,
                replica_groups=[list(range(NCORES))],
                ins=[ccw_in[:].opt()],
                outs=[ccw_out[:].opt()],
            )

            for fc in range(2, NFC):
                nc.sync.dma_start(wc_t[fc][:], wc_d.ap()[:, fc])
                nc.scalar.dma_start(ws_t[fc][:], ws_d.ap()[:, fc])
            nc.scalar.dma_start(melT_t[:], melT_d.ap())
            nc.gpsimd.dma_start(c2s[0][:, :, 520:], c2_d.ap()[0][:, :, 520:])
            nc.gpsimd.dma_start(c2rs[0][:, :, 520:], c2r_d.ap()[0][:, :, 520:])
            for b in range(1, BPC):
                nc.gpsimd.dma_start(c2s[b][:], c2_d.ap()[b])
                nc.gpsimd.dma_start(c2rs[b][:], c2r_d.ap()[b])

            # ---- pass 1: fold + folded DFT power + mel + log/affine ----
            slots = [(b, t0, tt) for b in range(BPC) for t0, tt in T_TILES]

            def emit_fold(si):
                # DVE fold: e/o m-chunks as adds/subs of shifted slices
                b, t0, tt = slots[si]
                e_t = sbeo.tile([128, NMC, tt], dt.bfloat16, tag="e")
                o_t = sbeo.tile([128, NMC, tt], dt.bfloat16, tag="o")
                for mc in range(NMC):
                    p1 = mc % 2
                    u1 = t0 + 4 + mc // 2
                    p2 = 1 - p1
                    u2 = t0 + 3 - mc // 2
                    a = c2s[b][:, p1, u1 : u1 + tt]
                    r = c2rs[b][:, p2, u2 : u2 + tt]
                    nc.vector.tensor_tensor(e_t[:, mc], a, r, ALU.add)
                    nc.vector.tensor_tensor(o_t[:, mc], a, r, ALU.subtract)
                return e_t, o_t

            eo_next = emit_fold(0)
            for slot, (b, t0, tt) in enumerate(slots):
                e_t, o_t = eo_next
                mel_ps = psM.tile([128, tt], dt.float32, tag="mel")
                for fc in range(NFC):
                    if fc == 4 and slot + 1 < len(slots):
                        # software-pipeline: fold the next slot's e/o now so
                        # the PE never waits on the DVE at slot boundaries
                        eo_next = emit_fold(slot + 1)
                    c_ps = psCS.tile([128, tt], dt.float32, tag="C")
                    s_ps = psCS.tile([128, tt], dt.float32, tag="S")
                    for mc in range(NMC):
                        nc.tensor.matmul(
                            c_ps[:], wc_t[fc][:, mc, :], e_t[:, mc],
                            start=(mc == 0), stop=(mc == NMC - 1),
                            skip_group_check=True,
                        )
                    for mc in range(NMC):
                        nc.tensor.matmul(
                            s_ps[:], ws_t[fc][:, mc, :], o_t[:, mc],
                            start=(mc == 0), stop=(mc == NMC - 1),
                            skip_group_check=True,
                        )
                    csq = sbe.tile([128, tt], dt.bfloat16, tag="csq")
                    ssq = sbe.tile([128, tt], dt.bfloat16, tag="ssq")
                    nc.scalar.activation(csq[:], c_ps[:], AF.Square)
                    nc.scalar.activation(ssq[:], s_ps[:], AF.Square)
                    if fc == 0:
                        # rank-1 repair of the Nyquist fold (see header)
                        nc.tensor.matmul(
                            mel_ps[:], melnyq_t[:], csq[0:1, :],
                            start=True, stop=False, skip_group_check=True,
                        )
                    stft = sbe.tile([128, tt], dt.bfloat16, tag="stft")
                    nc.vector.tensor_tensor(stft[:], csq[:], ssq[:], ALU.add)
                    nc.tensor.matmul(
                        mel_ps[:], melT_t[:, fc, :], stft[:],
                        start=False, stop=(fc == NFC - 1),
                        skip_group_check=True,
                    )
                mel_sb = sbe.tile([128, tt], dt.float32, tag="melsb")
                nc.vector.tensor_scalar(mel_sb[:], mel_ps[:], AMIN, None, ALU.max)
                nc.vector.tensor_reduce(
                    maxslots[:, slot : slot + 1], mel_sb[:],
                    mybir.AxisListType.X, ALU.max,
                )
                nc.scalar.activation(mel_sb[:], mel_sb[:], AF.Ln)
                nc.vector.tensor_scalar(
                    outp[b][:, t0 : t0 + tt], mel_sb[:],
                    C_LOG / 80.0, 25.0 / 80.0, ALU.mult, ALU.add,
                )

            # ---- local threshold, then AllReduce(max) of the threshold ----
            # The dB transform is monotone increasing, so
            # max_c f(lmax_c) == f(max_c lmax_c): compute the local o_thr
            # BEFORE the collective to keep the post-collective path minimal.
            lmax = sbw.tile([128, 1], dt.float32, name="lmax")
            nc.vector.tensor_reduce(
                lmax[:], maxslots[:], mybir.AxisListType.X, ALU.max
            )
            gmax = sbw.tile([128, 1], dt.float32, name="gmax")
            nc.gpsimd.partition_all_reduce(
                gmax[:], lmax[:], channels=128, reduce_op=bass_isa.ReduceOp.max
            )
            # o_thr_local = (C_LOG*ln(lmax*1e-8) + 25)/80, per-partition scalar
            thrlin = sbw.tile([128, 1], dt.float32, name="thrlin")
            nc.vector.tensor_scalar(thrlin[:], gmax[:], TOPDB_LIN, None, ALU.mult)
            thrln = sbw.tile([128, 1], dt.float32, name="thrln")
            nc.scalar.activation(thrln[:], thrlin[:], AF.Ln)
            lthr = sbw.tile([128, 1], dt.float32, name="lthr")
            nc.vector.tensor_scalar(
                lthr[:], thrln[:], C_LOG / 80.0, 25.0 / 80.0, ALU.mult, ALU.add
            )
            cc_in = dram.tile([1, 128], dt.float32, name="cc_in")
            cc_out = dram.tile([1, 128], dt.float32, name="cc_out")
            nc.sync.dma_start(cc_in[:], lthr[:])
            nc.gpsimd.collective_compute(
                "AllReduce",
                ALU.max,
                replica_groups=[list(range(NCORES))],
                ins=[cc_in[:].opt()],
                outs=[cc_out[:].opt()],
            )
            o_thr = sbw.tile([128, 1], dt.float32, name="o_thr")
            nc.sync.dma_start(o_thr[:], cc_out[:])

            # ---- fixup: out = max(out_pre, o_thr), in-place, then DMA out ----
            qs = [nc.sync, nc.scalar, nc.gpsimd]
            i = 0
            for b in range(BPC):
                for t0, tt in FIX_TILES:
                    nc.vector.tensor_scalar(
                        outp[b][:, t0 : t0 + tt], outp[b][:, t0 : t0 + tt],
                        o_thr[:], None, ALU.max,
                    )
                    qs[i % 3].dma_start(
                        out_d.ap()[b, :, t0 : t0 + tt], outp[b][:, t0 : t0 + tt]
                    )
                    i += 1

    nc.compile()
    return nc


def _get_nc():
    if "nc" not in _compiled:
        _compiled["nc"] = _build_nc()
    return _compiled["nc"]


def _prep_inputs(x, cos_w, sin_w, mel_w):
    x = np.asarray(x, dtype=np.float32).reshape(B, T)
    wcf = np.asarray(cos_w, dtype=np.float32).reshape(WIN // 2 + 1, WIN)  # [1025,2048]
    wsf = np.asarray(sin_w, dtype=np.float32).reshape(WIN // 2 + 1, WIN)
    mel = np.asarray(mel_w, dtype=np.float32)  # [128, 1025]

    # x -> [B, 128, 2, 864]: C2[r, par, u] = x[256u + 128par + r], bf16,
    # plus the partition-reversed copy for the fold's mirrored operand.
    x16 = x.astype(ml_dtypes.bfloat16)
    c2 = np.ascontiguousarray(x16.reshape(B, UCOLS, 2, 128).transpose(0, 3, 2, 1))
    c2r = np.ascontiguousarray(c2[:, ::-1])

    # Folded weights from the provided arrays via the phase rotation:
    #   cos_w[f, 1024+m] = w~ cos(theta k),  sin_w[f, 1024+m] = -w~ sin(theta k)
    #   (k = 1024+m = j + 1023.5), phi_f = 2 pi f 1023.5 / 2048
    #   W~c[m,f] = w~ cos(theta j) = cos(phi) cos_w + sin(phi) (-sin_w)... computed below
    f = np.arange(WIN // 2 + 1, dtype=np.float64)
    phi = 2.0 * np.pi * f * 1023.5 / WIN
    cph = np.cos(phi)[:, None]
    sph = np.sin(phi)[:, None]
    A = wcf[:, 1024:].astype(np.float64)  # [1025, 1024] = w~ cos(theta k)
    Bp = wsf[:, 1024:].astype(np.float64)  # = -w~ sin(theta k)
    Wc = cph * A - sph * Bp  # [f, m] = w~ cos(theta j)
    Ws = -(cph * Bp + sph * A)  # = w~ sin(theta j)
    # S column for f=0 is exactly zero; carry the Nyquist S row there
    Ws[0] = Ws[1024]
    Wc_use = Wc[:1024]  # [1024 f, 1024 m]
    Ws_use = Ws[:1024]

    def dev_w(Wfm):  # [1024 f, 1024 m] -> [128 p, NFC, NMC, 128 fi]
        a = Wfm.reshape(NFC, 128, NMC, 128)  # [fc, fi, mc, p]
        return np.ascontiguousarray(a.transpose(3, 0, 2, 1)).astype(
            ml_dtypes.bfloat16
        )

    wc_dev = dev_w(Wc_use)
    ws_dev = dev_w(Ws_use)

    # mel column for f=0 becomes mel_w[:,1024] (applied to C_0^2 + S_nyq^2);
    # the rank-1 (mel_w[:,0]-mel_w[:,1024]) x C_0^2 term repairs it
    mel_mod = mel[:, :1024].copy()
    mel_mod[:, 0] = mel[:, 1024]
    melT = np.ascontiguousarray(
        mel_mod.T.reshape(NFC, 128, NMEL).transpose(1, 0, 2)
    ).astype(ml_dtypes.bfloat16)  # [128 fi, NFC, NMEL]
    melnyq = np.ascontiguousarray((mel[:, 0] - mel[:, 1024])[None, :]).astype(
        ml_dtypes.bfloat16
    )  # [1, NMEL]
    return c2, c2r, wc_dev, ws_dev, melT, melnyq


def _make_in_maps(inputs):
    c2, c2r, wc_dev, ws_dev, melT, melnyq = _prep_inputs(**inputs)
    in_maps = []
    for c in range(NCORES):
        in_maps.append(
            {
                "c2": c2[c * BPC : (c + 1) * BPC],
                "c2r": c2r[c * BPC : (c + 1) * BPC],
                "wc": wc_dev,
                "ws": ws_dev,
                "melT": melT,
                "melnyq": melnyq,
            }
        )
    return in_maps


def kernel(x, cos_w, sin_w, mel_w):
    nc = _get_nc()
    in_maps = _make_in_maps(
        {"x": x, "cos_w": cos_w, "sin_w": sin_w, "mel_w": mel_w}
    )
    res = run_bass_kernel_spmd(nc, in_maps, list(range(NCORES)))
    out = np.concatenate([r["out"] for r in res.results], axis=0)  # [32,128,857]
    return out.astype(np.float32)


if __name__ == "__main__":
    rng = np.random.default_rng(0)
    x = rng.standard_normal((B, 1, T), dtype=np.float32)
    wc = rng.standard_normal((1025, 1, WIN), dtype=np.float32)
    wsn = rng.standard_normal((1025, 1, WIN), dtype=np.float32)
    mw = np.abs(rng.standard_normal((NMEL, 1025), dtype=np.float32)).astype(np.float32)
    o = kernel(x, wc, wsn, mw)
    print(o.shape, o.dtype)


# revision 10
# speedup vs baseline: 1.0492x; 1.0492x over previous
"""MelSpectrogramNet on 8 TRN2 NeuronCores (Bass/Tile), data-parallel over batch.

Math (per batch item):
  stft[f,t]  = (sum_k x[256t+k]*wc[f,k])^2 + (sum_k x[256t+k]*ws[f,k])^2
  mel        = mel_w @ stft
  x_db       = 10*log10(max(mel, 1e-10));  x_db = max(x_db, max_all(x_db)-80)
  out        = (x_db + 25) / 80

Folded DFT (the key trick): the hann window is exactly symmetric
(w[k] = w[2047-k]), so with j = k - 1023.5 the windowed DFT row is
w*cos(theta_f*j + phi_f). Folding x about the window center into
  e_m(t) = x[256t+1024+m] + x[256t+1023-m]
  o_m(t) = x[256t+1024+m] - x[256t+1023-m]        (m in [0,1024))
gives  cosDFT = cos(phi)C - sin(phi)S,  sinDFT = sin(phi)C + cos(phi)S with
  C_f = sum_m W~c[m,f] e_m,   S_f = sum_m W~s[m,f] o_m
and the power is phi-free:  power_f = C_f^2 + S_f^2.
=> the tensor-engine contraction halves (K=1024 per transform instead of
2x K=2048), which matters because the PE is the bottleneck (GPIO power
throttle caps it at 13/16 duty; the f32r version already ran at ~96% of
that throttled roofline).

Device mapping:
  - x is de-interleaved by 128-column parity into C2[r, par, u] =
    x[256u+128par+r] plus a partition-reversed copy C2R[r,...] =
    C2[127-r,...]; the DVE then computes each 128-row m-chunk of e/o as a
    single tensor_tensor add/sub of two contiguous slices (hidden under
    the matmuls of the previous tile).
  - all matmul operands are bf16 (measured end-to-end rel err ~5e-3 vs
    the 2e-2 gate); PSUM accumulation is fp32.
  - Nyquist: C_1024 = 0 exactly and the S-weight column for f=0 is exactly
    zero, so the S weights carry w~*(-1)^m (the Nyquist sine row) in the
    f=0 slot. Then stft[0] = C_0^2 + S_nyq^2; the mel weight column for
    f=0 is swapped to mel_w[:,1024] and a K=1 rank-1 matmul with
    (mel_w[:,0]-mel_w[:,1024]) x C_0^2 repairs the difference.
  - top_db clamp in linear space: pass 1 keeps out_pre in SBUF and the
    per-core max of mel; after gpsimd partition_all_reduce +
    AllReduce(max), the fixup applies out = max(out_pre, o_thr) in-place
    and DMAs straight out — no DRAM round-trip in the tail.
"""
import sys

sys.path.insert(0, "/opt/trn_rl_repo")

import ml_dtypes
import numpy as np

from concourse import bacc, bass_isa, mybir, tile
from concourse.bass_utils import run_bass_kernel_spmd

dt = mybir.dt
AF = mybir.ActivationFunctionType
ALU = mybir.AluOpType

NCORES = 8
B, T = 32, 221184
WIN, HOP = 2048, 256
FRAMES = (T - WIN) // HOP + 1  # 857
NMEL = 128
BPC = B // NCORES  # 4
UCOLS = T // 256  # 864 columns of 128 per parity
NFC = 8  # f-chunks of 128 (f = 0..1023); f=1024 (Nyquist) folded into S f=0
NMC = 8  # m-chunks of 128 (folded window half, m = 0..1023)
# Second tile overlaps the first by 3 frames so its width is a multiple of 4;
# overlapped frames are recomputed with identical values.
T_TILES = [(0, 512), (FRAMES - 348, 348)]
FIX_TILES = [(0, 512), (512, FRAMES - 512)]  # non-overlapping, for the fixup
C_LOG = 10.0 / float(np.log(10.0))  # 10*log10(x) = C_LOG * ln(x)
AMIN = 1e-10
TOPDB_LIN = 1e-8  # 10**(-80/10)

_compiled = {}


def _build_nc():
    nc = bacc.Bacc(
        "TRN2", target_bir_lowering=False, debug=False, num_devices=NCORES
    )

    c2_d = nc.dram_tensor("c2", [BPC, 128, 2, UCOLS], dt.bfloat16, kind="ExternalInput")
    c2r_d = nc.dram_tensor(
        "c2r", [BPC, 128, 2, UCOLS], dt.bfloat16, kind="ExternalInput"
    )
    wc_d = nc.dram_tensor("wc", [128, NFC, NMC, 128], dt.bfloat16, kind="ExternalInput")
    ws_d = nc.dram_tensor("ws", [128, NFC, NMC, 128], dt.bfloat16, kind="ExternalInput")
    melT_d = nc.dram_tensor("melT", [128, NFC, NMEL], dt.bfloat16, kind="ExternalInput")
    melnyq_d = nc.dram_tensor("melnyq", [1, NMEL], dt.bfloat16, kind="ExternalInput")
    out_d = nc.dram_tensor("out", [BPC, NMEL, FRAMES], dt.float32, kind="ExternalOutput")

    with tile.TileContext(nc) as tc:
        with (
            tc.tile_pool(name="sbw", bufs=1) as sbw,
            tc.tile_pool(name="sbeo", bufs=2) as sbeo,
            tc.tile_pool(name="sbe", bufs=3) as sbe,
            tc.tile_pool(name="psCS", bufs=3, space="PSUM") as psCS,
            tc.tile_pool(name="psM", bufs=2, space="PSUM") as psM,
            tc.tile_pool(name="dram", bufs=1, space="DRAM") as dram,
        ):
            # persistent SBUF tensors
            c2s, c2rs, outp = [], [], []
            for b in range(BPC):
                c2s.append(sbw.tile([128, 2, UCOLS], dt.bfloat16, name=f"c2_{b}"))
                c2rs.append(sbw.tile([128, 2, UCOLS], dt.bfloat16, name=f"c2r_{b}"))
                outp.append(sbw.tile([128, FRAMES], dt.float32, name=f"outp_{b}"))
            wc_t = [sbw.tile([128, NMC, 128], dt.bfloat16, name=f"wc{fc}") for fc in range(NFC)]
            ws_t = [sbw.tile([128, NMC, 128], dt.bfloat16, name=f"ws{fc}") for fc in range(NFC)]
            melT_t = sbw.tile([128, NFC, NMEL], dt.bfloat16, name="melT_t")
            melnyq_t = sbw.tile([1, NMEL], dt.bfloat16, name="melnyq_t")
            nslots = BPC * len(T_TILES)
            maxslots = sbw.tile([128, nslots], dt.float32, name="maxslots")

            # ---- input DMAs: b=0 slices needed by the first tile go first.
            # fold mc=0 needs c2 parity 0 + c2r parity 1, so those two land
            # first on separate queues.
            nc.gpsimd.dma_start(c2s[0][:, 0, 0:520], c2_d.ap()[0][:, 0, 0:520])
            nc.gpsimd.dma_start(c2rs[0][:, 1, 0:520], c2r_d.ap()[0][:, 1, 0:520])
            nc.gpsimd.dma_start(c2s[0][:, 1, 0:520], c2_d.ap()[0][:, 1, 0:520])
            nc.gpsimd.dma_start(c2rs[0][:, 0, 0:520], c2r_d.ap()[0][:, 0, 0:520])
            # fc=0/1 weights split across sync/scalar queues so the first
            # matmuls are never DMA-starved; melT's first chunks land early
            # (the mel matmul is on the in-order PE queue — starving it
            # stalls the PE), then the remaining f-chunks alternate queues.
            nc.sync.dma_start(wc_t[0][:, 0:4], wc_d.ap()[:, 0, 0:4])
            nc.scalar.dma_start(wc_t[0][:, 4:], wc_d.ap()[:, 0, 4:])
            nc.sync.dma_start(ws_t[0][:, 0:4], ws_d.ap()[:, 0, 0:4])
            nc.scalar.dma_start(ws_t[0][:, 4:], ws_d.ap()[:, 0, 4:])
            nc.sync.dma_start(melnyq_t[:], melnyq_d.ap())
            nc.sync.dma_start(melT_t[:, 0:2], melT_d.ap()[:, 0:2])
            nc.sync.dma_start(wc_t[1][:], wc_d.ap()[:, 1])
            nc.scalar.dma_start(ws_t[1][:], ws_d.ap()[:, 1])

            # Warm up the collective engine while the DFT runs so the real
            # AllReduce at the end starts with rings already configured.
            ccw_in = dram.tile([1, 128], dt.float32, name="ccw_in")
            ccw_out = dram.tile([1, 128], dt.float32, name="ccw_out")
            nc.gpsimd.collective_compute(
                "AllReduce",
                ALU.max,
                replica_groups=[list(range(NCORES))],
                ins=[ccw_in[:].opt()],
                outs=[ccw_out[:].opt()],
            )

            nc.sync.dma_start(wc_t[2][:], wc_d.ap()[:, 2])
            nc.scalar.dma_start(ws_t[2][:], ws_d.ap()[:, 2])
            nc.sync.dma_start(melT_t[:, 2:], melT_d.ap()[:, 2:])
            for fc in range(3, NFC):
                nc.sync.dma_start(wc_t[fc][:], wc_d.ap()[:, fc])
                nc.scalar.dma_start(ws_t[fc][:], ws_d.ap()[:, fc])
            nc.gpsimd.dma_start(c2s[0][:, :, 520:], c2_d.ap()[0][:, :, 520:])
            nc.gpsimd.dma_start(c2rs[0][:, :, 520:], c2r_d.ap()[0][:, :, 520:])
            for b in range(1, BPC):
                nc.gpsimd.dma_start(c2s[b][:], c2_d.ap()[b])
                nc.gpsimd.dma_start(c2rs[b][:], c2r_d.ap()[b])

            # ---- pass 1: fold + folded DFT power + mel + log/affine ----
            slots = [(b, t0, tt) for b in range(BPC) for t0, tt in T_TILES]
            # even m-chunks need (c2 par0, c2r par1); odds the other pair —
            # process evens first so the first matmuls match DMA arrival order
            MC_ORDER = [0, 2, 4, 6, 1, 3, 5, 7]

            def emit_fold(si):
                # DVE fold: e/o m-chunks as adds/subs of shifted slices
                b, t0, tt = slots[si]
                e_t = sbeo.tile([128, NMC, tt], dt.bfloat16, tag="e")
                o_t = sbeo.tile([128, NMC, tt], dt.bfloat16, tag="o")
                for mc in MC_ORDER:
                    p1 = mc % 2
                    u1 = t0 + 4 + mc // 2
                    p2 = 1 - p1
                    u2 = t0 + 3 - mc // 2
                    a = c2s[b][:, p1, u1 : u1 + tt]
                    r = c2rs[b][:, p2, u2 : u2 + tt]
                    nc.vector.tensor_tensor(e_t[:, mc], a, r, ALU.add)
                    nc.vector.tensor_tensor(o_t[:, mc], a, r, ALU.subtract)
                return e_t, o_t

            def emit_epilogue(slot):
                # mel -> clamp(AMIN) -> per-slot max -> ln -> affine -> outp
                b, t0, tt = slots[slot]
                mel_ps = mel_pss[slot]
                mel_sb = sbe.tile([128, tt], dt.float32, tag="melsb")
                nc.vector.tensor_scalar(mel_sb[:], mel_ps[:], AMIN, None, ALU.max)
                nc.vector.tensor_reduce(
                    maxslots[:, slot : slot + 1], mel_sb[:],
                    mybir.AxisListType.X, ALU.max,
                )
                nc.scalar.activation(mel_sb[:], mel_sb[:], AF.Ln)
                nc.vector.tensor_scalar(
                    outp[b][:, t0 : t0 + tt], mel_sb[:],
                    C_LOG / 80.0, 25.0 / 80.0, ALU.mult, ALU.add,
                )

            eo_next = emit_fold(0)
            mel_pss = {}
            for slot, (b, t0, tt) in enumerate(slots):
                e_t, o_t = eo_next
                mel_ps = psM.tile([128, tt], dt.float32, tag="mel")
                mel_pss[slot] = mel_ps
                # mel matmuls are emitted one fc-iteration late so the
                # in-order PE queue never waits on the Square/add chain;
                # (stft tile, fc) pending between iterations:
                pend = None
                for fc in range(NFC):
                    if fc == 4 and slot + 1 < len(slots):
                        # software-pipeline: fold the next slot's e/o now so
                        # the PE never waits on the DVE at slot boundaries
                        eo_next = emit_fold(slot + 1)
                    c_ps = psCS.tile([128, tt], dt.float32, tag="C")
                    s_ps = psCS.tile([128, tt], dt.float32, tag="S")
                    for i, mc in enumerate(MC_ORDER):
                        nc.tensor.matmul(
                            c_ps[:], wc_t[fc][:, mc, :], e_t[:, mc],
                            start=(i == 0), stop=(i == NMC - 1),
                            skip_group_check=True,
                        )
                    for i, mc in enumerate(MC_ORDER):
                        nc.tensor.matmul(
                            s_ps[:], ws_t[fc][:, mc, :], o_t[:, mc],
                            start=(i == 0), stop=(i == NMC - 1),
                            skip_group_check=True,
                        )
                    if fc == 1:
                        # rank-1 repair of the Nyquist fold (see header);
                        # first write of mel_ps (start=True)
                        nc.tensor.matmul(
                            mel_ps[:], melnyq_t[:], prev_csq[0:1, :],
                            start=True, stop=False, skip_group_check=True,
                        )
                    if pend is not None:
                        pstft, pfc = pend
                        nc.tensor.matmul(
                            mel_ps[:], melT_t[:, pfc, :], pstft[:],
                            start=False, stop=False, skip_group_check=True,
                        )
                    csq = sbe.tile([128, tt], dt.bfloat16, tag="csq")
                    ssq = sbe.tile([128, tt], dt.bfloat16, tag="ssq")
                    nc.scalar.activation(csq[:], c_ps[:], AF.Square)
                    nc.scalar.activation(ssq[:], s_ps[:], AF.Square)
                    if fc == 0:
                        prev_csq = csq
                    stft = sbe.tile([128, tt], dt.bfloat16, tag="stft")
                    nc.vector.tensor_tensor(stft[:], csq[:], ssq[:], ALU.add)
                    pend = (stft, fc)
                    if fc == 1 and slot > 0:
                        # previous slot's last mel matmul + epilogue, emitted
                        # here so its Square/add chain hides under this
                        # slot's DFT matmuls
                        lstft, lfc = last_pend
                        nc.tensor.matmul(
                            mel_pss[slot - 1][:], melT_t[:, lfc, :], lstft[:],
                            start=False, stop=True, skip_group_check=True,
                        )
                        emit_epilogue(slot - 1)
                last_pend = pend

            # last slot: flush the final mel matmul + epilogue directly
            lstft, lfc = last_pend
            nc.tensor.matmul(
                mel_pss[len(slots) - 1][:], melT_t[:, lfc, :], lstft[:],
                start=False, stop=True, skip_group_check=True,
            )
            emit_epilogue(len(slots) - 1)

            # ---- local threshold, then AllReduce(max) of the threshold ----
            # The dB transform is monotone increasing, so
            # max_c f(lmax_c) == f(max_c lmax_c): compute the local o_thr
            # BEFORE the collective to keep the post-collective path minimal.
            lmax = sbw.tile([128, 1], dt.float32, name="lmax")
            nc.vector.tensor_reduce(
                lmax[:], maxslots[:], mybir.AxisListType.X, ALU.max
            )
            gmax = sbw.tile([128, 1], dt.float32, name="gmax")
            nc.gpsimd.partition_all_reduce(
                gmax[:], lmax[:], channels=128, reduce_op=bass_isa.ReduceOp.max
            )
            # o_thr_local = (C_LOG*ln(lmax*1e-8) + 25)/80, per-partition scalar
            thrlin = sbw.tile([128, 1], dt.float32, name="thrlin")
            nc.vector.tensor_scalar(thrlin[:], gmax[:], TOPDB_LIN, None, ALU.mult)
            thrln = sbw.tile([128, 1], dt.float32, name="thrln")
            nc.scalar.activation(thrln[:], thrlin[:], AF.Ln)
            lthr = sbw.tile([128, 1], dt.float32, name="lthr")
            nc.vector.tensor_scalar(
                lthr[:], thrln[:], C_LOG / 80.0, 25.0 / 80.0, ALU.mult, ALU.add
            )
            cc_in = dram.tile([1, 128], dt.float32, name="cc_in")
            cc_out = dram.tile([1, 128], dt.float32, name="cc_out")
            nc.sync.dma_start(cc_in[:], lthr[:])
            nc.gpsimd.collective_compute(
                "AllReduce",
                ALU.max,
                replica_groups=[list(range(NCORES))],
                ins=[cc_in[:].opt()],
                outs=[cc_out[:].opt()],
            )
            o_thr = sbw.tile([128, 1], dt.float32, name="o_thr")
            nc.sync.dma_start(o_thr[:], cc_out[:])

            # ---- fixup: out = max(out_pre, o_thr), in-place, then DMA out ----
            qs = [nc.sync, nc.scalar, nc.gpsimd]
            i = 0
            for b in range(BPC):
                for t0, tt in FIX_TILES:
                    nc.vector.tensor_scalar(
                        outp[b][:, t0 : t0 + tt], outp[b][:, t0 : t0 + tt],
                        o_thr[:], None, ALU.max,
                    )
                    qs[i % 3].dma_start(
                        out_d.ap()[b, :, t0 : t0 + tt], outp[b][:, t0 : t0 + tt]
                    )
                    i += 1

    nc.compile()
    return nc


def _get_nc():
    if "nc" not in _compiled:
        _compiled["nc"] = _build_nc()
    return _compiled["nc"]


def _prep_inputs(x, cos_w, sin_w, mel_w):
    x = np.asarray(x, dtype=np.float32).reshape(B, T)
    wcf = np.asarray(cos_w, dtype=np.float32).reshape(WIN // 2 + 1, WIN)  # [1025,2048]
    wsf = np.asarray(sin_w, dtype=np.float32).reshape(WIN // 2 + 1, WIN)
    mel = np.asarray(mel_w, dtype=np.float32)  # [128, 1025]

    # x -> [B, 128, 2, 864]: C2[r, par, u] = x[256u + 128par + r], bf16,
    # plus the partition-reversed copy for the fold's mirrored operand.
    x16 = x.astype(ml_dtypes.bfloat16)
    c2 = np.ascontiguousarray(x16.reshape(B, UCOLS, 2, 128).transpose(0, 3, 2, 1))
    c2r = np.ascontiguousarray(c2[:, ::-1])

    # Folded weights from the provided arrays via the phase rotation:
    #   cos_w[f, 1024+m] = w~ cos(theta k),  sin_w[f, 1024+m] = -w~ sin(theta k)
    #   (k = 1024+m = j + 1023.5), phi_f = 2 pi f 1023.5 / 2048
    #   W~c[m,f] = w~ cos(theta j) = cos(phi) cos_w + sin(phi) (-sin_w)... computed below
    f = np.arange(WIN // 2 + 1, dtype=np.float64)
    phi = 2.0 * np.pi * f * 1023.5 / WIN
    cph = np.cos(phi)[:, None]
    sph = np.sin(phi)[:, None]
    A = wcf[:, 1024:].astype(np.float64)  # [1025, 1024] = w~ cos(theta k)
    Bp = wsf[:, 1024:].astype(np.float64)  # = -w~ sin(theta k)
    Wc = cph * A - sph * Bp  # [f, m] = w~ cos(theta j)
    Ws = -(cph * Bp + sph * A)  # = w~ sin(theta j)
    # S column for f=0 is exactly zero; carry the Nyquist S row there
    Ws[0] = Ws[1024]
    Wc_use = Wc[:1024]  # [1024 f, 1024 m]
    Ws_use = Ws[:1024]

    def dev_w(Wfm):  # [1024 f, 1024 m] -> [128 p, NFC, NMC, 128 fi]
        a = Wfm.reshape(NFC, 128, NMC, 128)  # [fc, fi, mc, p]
        return np.ascontiguousarray(a.transpose(3, 0, 2, 1)).astype(
            ml_dtypes.bfloat16
        )

    wc_dev = dev_w(Wc_use)
    ws_dev = dev_w(Ws_use)

    # mel column for f=0 becomes mel_w[:,1024] (applied to C_0^2 + S_nyq^2);
    # the rank-1 (mel_w[:,0]-mel_w[:,1024]) x C_0^2 term repairs it
    mel_mod = mel[:, :1024].copy()
    mel_mod[:, 0] = mel[:, 1024]
    melT = np.ascontiguousarray(
        mel_mod.T.reshape(NFC, 128, NMEL).transpose(1, 0, 2)
    ).astype(ml_dtypes.bfloat16)  # [128 fi, NFC, NMEL]
    melnyq = np.ascontiguousarray((mel[:, 0] - mel[:, 1024])[None, :]).astype(
        ml_dtypes.bfloat16
    )  # [1, NMEL]
    return c2, c2r, wc_dev, ws_dev, melT, melnyq


def _make_in_maps(inputs):
    c2, c2r, wc_dev, ws_dev, melT, melnyq = _prep_inputs(**inputs)
    in_maps = []
    for c in range(NCORES):
        in_maps.append(
            {
                "c2": c2[c * BPC : (c + 1) * BPC],
                "c2r": c2r[c * BPC : (c + 1) * BPC],
                "wc": wc_dev,
                "ws": ws_dev,
                "melT": melT,
                "melnyq": melnyq,
            }
        )
    return in_maps


def kernel(x, cos_w, sin_w, mel_w):
    nc = _get_nc()
    in_maps = _make_in_maps(
        {"x": x, "cos_w": cos_w, "sin_w": sin_w, "mel_w": mel_w}
    )
    res = run_bass_kernel_spmd(nc, in_maps, list(range(NCORES)))
    out = np.concatenate([r["out"] for r in res.results], axis=0)  # [32,128,857]
    return out.astype(np.float32)


if __name__ == "__main__":
    rng = np.random.default_rng(0)
    x = rng.standard_normal((B, 1, T), dtype=np.float32)
    wc = rng.standard_normal((1025, 1, WIN), dtype=np.float32)
    wsn = rng.standard_normal((1025, 1, WIN), dtype=np.float32)
    mw = np.abs(rng.standard_normal((NMEL, 1025), dtype=np.float32)).astype(np.float32)
    o = kernel(x, wc, wsn, mw)
    print(o.shape, o.dtype)
